# revision 15
# baseline (speedup 1.0000x reference)
"""Trainium2 Bass kernel for nn_BankedDenoiser.

Data-parallel over batch: 8 batch elements -> 8 NeuronCores, one element per
core, no collectives. Activations are kept feature-major (hT [D, S]) in SBUF so
every y = x @ W becomes matmul(lhsT=W_chunk, rhs=hT) with weights in their
natural DRAM layout. Matmuls run in bf16 with fp32 PSUM accumulation.

Attention: scores are produced transposed (scoresT [key, query]) so the
exponentiated weights can be used directly as the A@V matmul rhs; the softmax
denominator comes from augmenting token-major V with a ones column (PSUM row
64), and the per-query normalization is applied via a rank-1 ones-broadcast
matmul of the reciprocal row. exp() runs without max-subtraction (scores are
bounded for this module's weight scale) straight out of PSUM on ScalarE.

LayerNorm (feature-major): sum / sum-of-squares via ones-column matmuls on the
PE, rstd = exp(-0.5 * ln(var/D + eps)) so the whole kernel only needs the
natural_log_exp activation table set; mean/rstd rows broadcast with rank-1
matmuls.

Router top-4: Max8 gives each token's top-8 logits sorted; the dense dispatch
matrix P^T[m, s] = exp(logit - m1 - ln(sum_top4_exp)) * (logit >= (m4+m5)/2)
is built in transposed layout and applied as a matmul against token-major Z.
"""

import numpy as np
import ml_dtypes

B, S, IN_DIM, D, H, L, M, TOPK = 8, 1024, 256, 512, 8, 4, 256, 4
DFF = 2048
DH = D // H
TAU, GAMMA, BETA, ETA = 1.0, 0.3, 1.0, 1.0
P = 128
KD = D // P          # 4 feature chunks of 128
SC = S // 512        # 2 column chunks of 512
SCH = S // P         # 8 token chunks of 128
NF = DFF // P        # 16 dff chunks
C2 = BETA / (TAU * D)

_CACHE = {}


def _build(flags, reps=1):
    import concourse.tile as tile
    from concourse import bacc, mybir
    from concourse.masks import make_identity

    f32 = mybir.dt.float32
    bf16 = mybir.dt.bfloat16
    AF = mybir.ActivationFunctionType
    OP = mybir.AluOpType
    AX = mybir.AxisListType

    assert not flags["mask"], "non-trivial mask not supported"
    assert not flags["bqkv"], "nonzero enc qkv bias not supported"

    nc = bacc.Bacc("TRN2", target_bir_lowering=False, debug=False, num_devices=8)

    def din(name, shape, dt=f32):
        return nc.dram_tensor(name, shape, dt, kind="ExternalInput").ap()

    xT_d = din("xT", [IN_DIM, S])
    temb_d = din("temb", [D])           # t_embed[b] + b_in (host-folded)
    peT_d = din("peT", [D, S], bf16)
    phiT_d = din("phiT", [D, M])
    phi_d = din("phi", [M, D])
    sig_d = din("sig", [M, D])
    size_d = din("size2", [P, M // P])
    win_d = din("win", [IN_DIM, D])
    wqkv_d = din("wqkv", [L, D, 3 * D])
    wo_d = din("wo4", [L, D, D])
    w1_d = din("w1", [L, D, DFF])
    w2_d = din("w2", [L, DFF, D])
    saq_d = din("saq", [D, D])
    sak_d = din("sak", [D, D])
    sav_d = din("sav", [D, D])
    sao_d = din("sao", [D, D])
    rtq_d = din("rtq", [D, D])
    wout_d = din("wout", [D, IN_DIM])
    bo_d = din("bo4", [L, D]) if flags["bo"] else None
    b1_d = din("b14", [L, DFF]) if flags["b1"] else None
    b2_d = din("b24", [L, D]) if flags["b2"] else None
    ln_d = din("lnp", [L, 4, D]) if flags["ln"] else None
    bout_d = din("bout", [IN_DIM]) if flags["bout"] else None
    out_d = nc.dram_tensor("outT", [IN_DIM, S], f32, kind="ExternalOutput").ap()

    with tile.TileContext(nc) as tc:
        with (tc.tile_pool(name="const", bufs=1) as cpool,
              tc.tile_pool(name="stage", bufs=2) as stage,
              tc.tile_pool(name="keep", bufs=1) as keep,
              tc.tile_pool(name="psmm", bufs=3, space="PSUM") as ps_mm,
              tc.tile_pool(name="psav", bufs=2, space="PSUM") as ps_av,
              tc.tile_pool(name="pssm", bufs=2, space="PSUM") as ps_sm,
              tc.tile_pool(name="dram", bufs=2, space="DRAM") as dpool):

            ident = cpool.tile([P, P], f32)
            make_identity(nc, ident[:])
            ones_r = cpool.tile([1, D], f32)
            nc.vector.memset(ones_r[:], 1.0)
            ones_cf = cpool.tile([P, 1], f32)
            nc.vector.memset(ones_cf[:], 1.0)
            ones_cb = cpool.tile([P, 1], bf16)
            nc.vector.memset(ones_cb[:], 1.0)
            eps6_c = cpool.tile([P, 1], f32)
            nc.vector.memset(eps6_c[:], 1e-6)
            eps5_r = cpool.tile([1, 1], f32)
            nc.vector.memset(eps5_r[:], 1e-5)

            _pmm_n = [0]

            def pmm():
                _pmm_n[0] += 1
                return ps_mm.tile([P, 512], f32, tag="mm", bufs=4,
                                  name=f"mmps{_pmm_n[0]}")

            # DRAM fp32 [K, N] -> SBUF bf16 [P, K//P, N] (via fp32 staging)
            def load_w(pool, dram2d, K, N, tag, scale=None):
                ko_n = K // P
                chunk = max(1, min(ko_n, 2048 // N))
                w = pool.tile([P, ko_n, N], bf16, tag=tag)
                src = dram2d.rearrange("(ko p) m -> p ko m", p=P)
                for c0 in range(0, ko_n, chunk):
                    cn = min(chunk, ko_n - c0)
                    st = stage.tile([P, 2048], f32, tag="stage", bufs=2)
                    stv = st[:, :cn * N].rearrange("p (c m) -> p c m", c=cn)
                    nc.sync.dma_start(stv, src[:, c0:c0 + cn, :])
                    if scale is None:
                        nc.gpsimd.tensor_copy(w[:, c0:c0 + cn, :], stv)
                    else:
                        nc.gpsimd.tensor_scalar_mul(w[:, c0:c0 + cn, :], stv, scale)
                return w

            def col_from(dram1d, n, tag):
                t = cpool.tile([P, n // P], f32, tag=tag)
                nc.sync.dma_start(t[:], dram1d.rearrange("(o p) -> p o", p=P))
                return t

            if flags["bo"]:
                bo_c = [col_from(bo_d[l], D, f"bo{l}") for l in range(L)]
            if flags["b1"]:
                b1_c = [col_from(b1_d[l], DFF, f"b1{l}") for l in range(L)]
            if flags["b2"]:
                b2_c = [col_from(b2_d[l], D, f"b2{l}") for l in range(L)]
            if flags["ln"]:
                ln_c = [[col_from(ln_d[l, j], D, f"ln{l}_{j}") for j in range(4)]
                        for l in range(L)]
            if flags["bout"]:
                bout_c = col_from(bout_d, IN_DIM, "boutc")
            temb_c = col_from(temb_d, D, "tembc")

            # persistent across phases
            phiT_b = keep.tile([P, KD, M], bf16, tag="phiTb")
            z_sb = keep.tile([P, 2, D], bf16, tag="ztok")

            def bank_phase(bp):
                saq_w = load_w(bp, saq_d, D, D, "saq")
                sak_w = load_w(bp, sak_d, D, D, "sak",
                               scale=BETA * ETA / np.sqrt(DH))
                sav_w = load_w(bp, sav_d, D, D, "sav")
                sao_w = load_w(bp, sao_d, D, D, "sao")

                st = stage.tile([P, 2048], f32, tag="stage", bufs=2)
                phiT_f = st[:, :KD * M].rearrange("p (ko m) -> p ko m", ko=KD)
                nc.sync.dma_start(phiT_f, phiT_d.rearrange("(ko p) m -> p ko m", p=P))
                nc.gpsimd.tensor_copy(phiT_b[:], phiT_f)
                phiT_2c = bp.tile([P, KD, M], bf16, tag="phiT2c")
                nc.gpsimd.tensor_scalar_mul(phiT_2c[:], phiT_f, 2.0 * C2)

                phi_sb = bp.tile([P, 2, D], f32, tag="phitok")
                nc.sync.dma_start(phi_sb[:], phi_d.rearrange("(c p) d -> p c d", p=P))
                sig_sb = bp.tile([P, 2, D], f32, tag="sigtok")
                nc.sync.dma_start(sig_sb[:], sig_d.rearrange("(c p) d -> p c d", p=P))
                size_sb = bp.tile([P, 2], f32, tag="sizec")
                nc.sync.dma_start(size_sb[:], size_d[:])

                p2_c = bp.tile([P, 2], f32, tag="p2c")
                sig_c = bp.tile([P, 2], f32, tag="sigc")
                for c in range(2):
                    sq = bp.tile([P, D], bf16, tag="banksq", bufs=2)
                    nc.scalar.activation(sq[:], phi_sb[:, c, :], AF.Square)
                    nc.vector.reduce_sum(p2_c[:, c:c + 1], sq[:], axis=AX.X)
                    sq2 = bp.tile([P, D], bf16, tag="banksq", bufs=2)
                    nc.scalar.activation(sq2[:], sig_sb[:, c, :], AF.Square)
                    nc.vector.reduce_sum(sig_c[:, c:c + 1], sq2[:], axis=AX.X)
                lnsz = bp.tile([P, 2], f32, tag="lnsz")
                nc.scalar.activation(lnsz[:], size_sb[:], AF.Ln, bias=eps6_c[:])
                d_col = bp.tile([P, 2], f32, tag="dcol")
                t_col = bp.tile([P, 2], f32, tag="tcol")
                nc.vector.tensor_scalar_mul(d_col[:], lnsz[:], GAMMA)
                nc.vector.tensor_scalar_mul(t_col[:], sig_c[:], 0.5 / D)
                nc.vector.tensor_sub(d_col[:], d_col[:], t_col[:])
                nc.vector.tensor_scalar_mul(t_col[:], p2_c[:], C2)
                nc.vector.tensor_sub(d_col[:], d_col[:], t_col[:])
                p2m_c = bp.tile([P, 2], f32, tag="p2mc")
                nc.vector.tensor_scalar_mul(p2m_c[:], p2_c[:], -C2)
                pack = bp.tile([P, 4], f32, tag="pack4")
                nc.vector.tensor_copy(pack[:, 0:2], d_col[:])
                nc.vector.tensor_copy(pack[:, 2:4], p2m_c[:])
                tp_ps = ps_sm.tile([16, 512], f32, tag="small", bufs=2)
                nc.tensor.transpose(tp_ps[:4, :P], pack[:], ident[:])
                t4 = bp.tile([4, P], f32, tag="t4sb")
                nc.vector.tensor_copy(t4[:], tp_ps[:4, :P])
                dsc = dpool.tile([16, P], f32, tag="dscr")
                nc.sync.dma_start(dsc[0:4, :], t4[:])
                d_row = bp.tile([1, M], f32, tag="drow")
                p2m_row = bp.tile([1, M], f32, tag="p2mrow")
                for c in range(2):
                    nc.sync.dma_start(d_row[:, c * P:(c + 1) * P], dsc[c:c + 1, :])
                    nc.sync.dma_start(p2m_row[:, c * P:(c + 1) * P],
                                      dsc[2 + c:3 + c, :])

                qTb = bp.tile([P, KD, M], bf16, tag="qTb")
                kTb = bp.tile([P, KD, M], bf16, tag="kTb")
                for dst, wmat in ((qTb, saq_w), (kTb, sak_w)):
                    for m in range(KD):
                        ps = pmm()
                        for k in range(KD):
                            nc.tensor.matmul(ps[:, :M],
                                             wmat[:, k, m * P:(m + 1) * P],
                                             phiT_b[:, k, :],
                                             start=(k == 0), stop=(k == KD - 1))
                        nc.vector.tensor_copy(dst[:, m, :], ps[:, :M])
                vb_aug = bp.tile([P, 2, H, DH + 1], bf16, tag="vbaug")
                nc.vector.memset(vb_aug[:], 1.0)
                for nch in range(2):
                    ps = pmm()
                    for k in range(KD):
                        nc.tensor.matmul(ps[:], phiT_b[:, k, nch * P:(nch + 1) * P],
                                         sav_w[:, k, :],
                                         start=(k == 0), stop=(k == KD - 1))
                    nc.vector.tensor_copy(
                        vb_aug[:, nch, :, 0:DH],
                        ps[:].rearrange("p (h c) -> p h c", c=DH))

                oTb = bp.tile([P, KD, M], bf16, tag="oTb")
                for h in range(H):
                    p0, ko = DH * (h % 2), h // 2
                    eb = bp.tile([P, 2, M], bf16, tag="expb", bufs=2)
                    for nch in range(2):
                        ps = pmm()
                        for k in range(KD):
                            nc.tensor.matmul(ps[:, :M],
                                             phiT_b[:, k, nch * P:(nch + 1) * P],
                                             phiT_2c[:, k, :],
                                             start=(k == 0), stop=False)
                        nc.tensor.matmul(ps[:, :M], d_row[:, nch * P:(nch + 1) * P],
                                         ones_r[:, :M], start=False, stop=False)
                        nc.tensor.matmul(ps[:, :M], ones_r[:, :P], p2m_row[:],
                                         start=False, stop=False)
                        nc.tensor.matmul(ps[:, :M],
                                         kTb[p0:p0 + DH, ko, nch * P:(nch + 1) * P],
                                         qTb[p0:p0 + DH, ko, :],
                                         start=False, stop=True)
                        nc.scalar.activation(eb[:, nch, :], ps[:, :M], AF.Exp)
                    zb = ps_av.tile([DH + 1, 512], f32, tag="av", bufs=2)
                    for nch in range(2):
                        nc.tensor.matmul(zb[:, :M], vb_aug[:, nch, h, :],
                                         eb[:, nch, :],
                                         start=(nch == 0), stop=(nch == 1))
                    den = bp.tile([1, M], f32, tag="denb", bufs=2)
                    nc.vector.tensor_copy(den[:], zb[DH:DH + 1, :M])
                    rb = bp.tile([1, M], f32, tag="recb", bufs=2)
                    nc.vector.reciprocal(rb[:], den[:])
                    bc = ps_sm.tile([DH, 512], f32, tag="small", bufs=2)
                    nc.tensor.matmul(bc[:, :M], ones_r[:, :DH], rb[:],
                                     start=True, stop=True)
                    bcs = bp.tile([DH, M], bf16, tag="bcsb", bufs=2)
                    nc.vector.tensor_copy(bcs[:], bc[:, :M])
                    nc.vector.tensor_mul(oTb[p0:p0 + DH, ko, :], zb[0:DH, :M], bcs[:])
                for mch in range(2):
                    ps = pmm()
                    for k in range(KD):
                        nc.tensor.matmul(ps[:], oTb[:, k, mch * P:(mch + 1) * P],
                                         sao_w[:, k, :],
                                         start=(k == 0), stop=(k == KD - 1))
                    nc.vector.tensor_copy(z_sb[:, mch, :], ps[:])

            def inproj_phase(ip):
                win_w = load_w(ip, win_d, IN_DIM, D, "win")
                peT_sb = ip.tile([P, KD, S], bf16, tag="peT")
                nc.sync.dma_start(peT_sb[:],
                                  peT_d.rearrange("(ko p) s -> p ko s", p=P))
                st = stage.tile([P, 2048], f32, tag="stage", bufs=2)
                xT_f = st[:, :2 * 1024].rearrange("p (ko s) -> p ko s", ko=2)
                nc.sync.dma_start(xT_f, xT_d.rearrange("(ko p) s -> p ko s", p=P))
                xT_b = ip.tile([P, 2, S], bf16, tag="xTb")
                nc.gpsimd.tensor_copy(xT_b[:], xT_f)
                h_sb = keep.tile([P, KD, S], bf16, tag="hT", bufs=2)
                for m in range(KD):
                    for sc in range(SC):
                        sl = slice(sc * 512, (sc + 1) * 512)
                        ps = pmm()
                        for k in range(2):
                            nc.tensor.matmul(ps[:], win_w[:, k, m * P:(m + 1) * P],
                                             xT_b[:, k, sl],
                                             start=(k == 0), stop=(k == 1))
                        tmp = ip.tile([P, 512], bf16, tag="iptmp", bufs=2)
                        nc.vector.tensor_add(tmp[:], ps[:], peT_sb[:, m, sl])
                        nc.vector.tensor_scalar_add(h_sb[:, m, sl], tmp[:],
                                                    temb_c[:, m:m + 1])
                return h_sb

            def emit_ln(ep, r_t, rsq_t, lidx, lnoff):
                # r_t: bf16 [P, KD, S]; rsq_t: bf16 [P, KD, S] (squares)
                out = keep.tile([P, KD, S], bf16, tag="hT", bufs=2)
                for sc in range(SC):
                    sl = slice(sc * 512, (sc + 1) * 512)
                    ps1 = ps_sm.tile([16, 512], f32, tag="small", bufs=2)
                    for k in range(KD):
                        nc.tensor.matmul(ps1[:1, :], ones_cb[:], r_t[:, k, sl],
                                         start=(k == 0), stop=(k == KD - 1))
                    ps2 = ps_sm.tile([16, 512], f32, tag="small", bufs=2)
                    for k in range(KD):
                        nc.tensor.matmul(ps2[:1, :], ones_cb[:], rsq_t[:, k, sl],
                                         start=(k == 0), stop=(k == KD - 1))
                    mu_row = ep.tile([1, 512], f32, tag="murow", bufs=2)
                    nc.vector.tensor_scalar_mul(mu_row[:], ps1[:1, :], 1.0 / D)
                    tr = ep.tile([1, 512], f32, tag="tmprow", bufs=2)
                    nc.vector.tensor_mul(tr[:], ps1[:1, :], mu_row[:])
                    var_row = ep.tile([1, 512], f32, tag="varrow", bufs=2)
                    nc.vector.tensor_sub(var_row[:], ps2[:1, :], tr[:])
                    nc.scalar.activation(var_row[:], var_row[:], AF.Ln,
                                         bias=eps5_r[:], scale=1.0 / D)
                    rstd_row = ep.tile([1, 512], f32, tag="rstdrow", bufs=2)
                    nc.scalar.activation(rstd_row[:], var_row[:], AF.Exp, scale=-0.5)
                    mb_ps = pmm()
                    nc.tensor.matmul(mb_ps[:], ones_r[:, :P], mu_row[:],
                                     start=True, stop=True)
                    mb = ep.tile([P, 512], bf16, tag="mubc", bufs=2)
                    nc.vector.tensor_copy(mb[:], mb_ps[:])
                    rb_ps = pmm()
                    nc.tensor.matmul(rb_ps[:], ones_r[:, :P], rstd_row[:],
                                     start=True, stop=True)
                    rbt = ep.tile([P, 512], bf16, tag="rstdbc", bufs=2)
                    nc.vector.tensor_copy(rbt[:], rb_ps[:])
                    for k in range(KD):
                        t1 = ep.tile([P, 512], bf16, tag="lnt1", bufs=2)
                        nc.vector.tensor_sub(t1[:], r_t[:, k, sl], mb[:])
                        if flags["ln"]:
                            t2 = ep.tile([P, 512], bf16, tag="lnt2", bufs=2)
                            nc.vector.tensor_mul(t2[:], t1[:], rbt[:])
                            nc.vector.tensor_scalar(
                                out[:, k, sl], t2[:],
                                ln_c[lidx][lnoff][:, k:k + 1],
                                ln_c[lidx][lnoff + 1][:, k:k + 1], OP.mult, OP.add)
                        else:
                            nc.vector.tensor_mul(out[:, k, sl], t1[:], rbt[:])
                return out

            def encoder_layer(ep, l, h_sb):
                wqkv_w = load_w(ep, wqkv_d[l], D, 3 * D, "wqkv")
                wo_w = load_w(ep, wo_d[l], D, D, "wo")
                w1_w = load_w(ep, w1_d[l], D, DFF, "w1")
                w2_w = load_w(ep, w2_d[l], DFF, D, "w2")
                qT = ep.tile([P, KD, S], bf16, tag="qT")
                kT = ep.tile([P, KD, S], bf16, tag="kT")
                for which, dst in ((0, qT), (1, kT)):
                    off = which * D
                    for m in range(KD):
                        for sc in range(SC):
                            sl = slice(sc * 512, (sc + 1) * 512)
                            ps = pmm()
                            for k in range(KD):
                                nc.tensor.matmul(
                                    ps[:], wqkv_w[:, k, off + m * P:off + (m + 1) * P],
                                    h_sb[:, k, sl],
                                    start=(k == 0), stop=(k == KD - 1))
                            if which == 0:
                                nc.vector.tensor_scalar_mul(dst[:, m, sl], ps[:],
                                                            1.0 / np.sqrt(DH))
                            else:
                                nc.scalar.copy(dst[:, m, sl], ps[:])
                v_aug = ep.tile([P, SCH, H, DH + 1], bf16, tag="vaug")
                nc.vector.memset(v_aug[:], 1.0)
                for tch in range(SCH):
                    ps = pmm()
                    for k in range(KD):
                        nc.tensor.matmul(ps[:], h_sb[:, k, tch * P:(tch + 1) * P],
                                         wqkv_w[:, k, 2 * D:3 * D],
                                         start=(k == 0), stop=(k == KD - 1))
                    nc.scalar.copy(
                        v_aug[:, tch, :, 0:DH],
                        ps[:].rearrange("p (h c) -> p h c", c=DH))
                oT = ep.tile([P, KD, S], bf16, tag="oT")
                # heads 2*ko (partitions 0:64) and 2*ko+1 (64:128) interleave so
                # their K=64 score matmuls pack into disjoint PE row groups
                for ko in range(KD):
                    for sc in range(SC):
                        sl = slice(sc * 512, (sc + 1) * 512)
                        ets = [[], []]
                        for tch in range(SCH):
                            for hp in range(2):
                                p0 = DH * hp
                                ps = pmm()
                                nc.tensor.matmul(
                                    ps[:], kT[p0:p0 + DH, ko, tch * P:(tch + 1) * P],
                                    qT[p0:p0 + DH, ko, sl], start=True, stop=True)
                                et = ep.tile([P, 512], bf16, tag="expT", bufs=20)
                                nc.scalar.activation(et[:], ps[:], AF.Exp)
                                ets[hp].append(et)
                        for hp in range(2):
                            h = 2 * ko + hp
                            p0 = DH * hp
                            zo = ps_av.tile([DH + 1, 512], f32, tag="av", bufs=2)
                            for tch in range(SCH):
                                nc.tensor.matmul(zo[:], v_aug[:, tch, h, :],
                                                 ets[hp][tch][:],
                                                 start=(tch == 0), stop=(tch == SCH - 1))
                            den = ep.tile([1, 512], f32, tag="den", bufs=2)
                            nc.vector.tensor_copy(den[:], zo[DH:DH + 1, :])
                            rcp = ep.tile([1, 512], f32, tag="rcp", bufs=2)
                            nc.vector.reciprocal(rcp[:], den[:])
                            bc = ps_sm.tile([DH, 512], f32, tag="small", bufs=2)
                            nc.tensor.matmul(bc[:], ones_r[:, :DH], rcp[:],
                                             start=True, stop=True)
                            bcs = ep.tile([DH, 512], bf16, tag="bcs", bufs=2)
                            nc.vector.tensor_copy(bcs[:], bc[:])
                            nc.vector.tensor_mul(oT[p0:p0 + DH, ko, sl],
                                                 zo[0:DH, :], bcs[:])
                r_t = ep.tile([P, KD, S], bf16, tag="resid")
                rsq_t = ep.tile([P, KD, S], bf16, tag="rsq")
                for m in range(KD):
                    for sc in range(SC):
                        sl = slice(sc * 512, (sc + 1) * 512)
                        ps = pmm()
                        for k in range(KD):
                            nc.tensor.matmul(ps[:], wo_w[:, k, m * P:(m + 1) * P],
                                             oT[:, k, sl],
                                             start=(k == 0), stop=(k == KD - 1))
                        if flags["bo"]:
                            nc.vector.tensor_scalar_add(ps[:], ps[:], bo_c[l][:, m:m + 1])
                        nc.vector.tensor_add(r_t[:, m, sl], ps[:], h_sb[:, m, sl])
                        nc.scalar.activation(rsq_t[:, m, sl], r_t[:, m, sl], AF.Square)
                h_sb = emit_ln(ep, r_t, rsq_t, l, 0)
                r_t = ep.tile([P, KD, S], bf16, tag="resid")
                rsq_t = ep.tile([P, KD, S], bf16, tag="rsq")
                for sc in range(SC):
                    sl = slice(sc * 512, (sc + 1) * 512)
                    ff = ep.tile([P, NF, 512], bf16, tag="ffT")
                    for m in range(NF):
                        ps = pmm()
                        for k in range(KD):
                            nc.tensor.matmul(ps[:], w1_w[:, k, m * P:(m + 1) * P],
                                             h_sb[:, k, sl],
                                             start=(k == 0), stop=(k == KD - 1))
                        if flags["b1"]:
                            nc.vector.tensor_scalar(ff[:, m, :], ps[:],
                                                    b1_c[l][:, m:m + 1], 0.0,
                                                    OP.add, OP.max)
                        else:
                            nc.vector.tensor_scalar_max(ff[:, m, :], ps[:], 0.0)
                    for m in range(KD):
                        ps = pmm()
                        for k in range(NF):
                            nc.tensor.matmul(ps[:], w2_w[:, k, m * P:(m + 1) * P],
                                             ff[:, k, :],
                                             start=(k == 0), stop=(k == NF - 1))
                        if flags["b2"]:
                            nc.vector.tensor_scalar_add(ps[:], ps[:], b2_c[l][:, m:m + 1])
                        nc.vector.tensor_add(r_t[:, m, sl], ps[:], h_sb[:, m, sl])
                        nc.scalar.activation(rsq_t[:, m, sl], r_t[:, m, sl], AF.Square)
                return emit_ln(ep, r_t, rsq_t, l, 2)

            def router_phase(rp, h_sb):
                rtq_w = load_w(rp, rtq_d, D, D, "rtq", scale=1.0 / np.sqrt(D))
                wout_w = load_w(rp, wout_d, D, IN_DIM, "wout")
                qrT = rp.tile([P, KD, S], bf16, tag="qrT")
                for m in range(KD):
                    for sc in range(SC):
                        sl = slice(sc * 512, (sc + 1) * 512)
                        ps = pmm()
                        for k in range(KD):
                            nc.tensor.matmul(ps[:], rtq_w[:, k, m * P:(m + 1) * P],
                                             h_sb[:, k, sl],
                                             start=(k == 0), stop=(k == KD - 1))
                        nc.vector.tensor_copy(qrT[:, m, sl], ps[:])
                pk = rp.tile([P, 16], f32, tag="pk")
                for sch in range(SCH):
                    ps = pmm()
                    for k in range(KD):
                        nc.tensor.matmul(ps[:, :M], qrT[:, k, sch * P:(sch + 1) * P],
                                         phiT_b[:, k, :],
                                         start=(k == 0), stop=(k == KD - 1))
                    lg = rp.tile([P, M], f32, tag="lgtok", bufs=2)
                    nc.vector.tensor_copy(lg[:], ps[:, :M])
                    mx = rp.tile([P, 8], f32, tag="mx8", bufs=2)
                    nc.vector.max(mx[:], lg[:])
                    e4 = rp.tile([P, 4], f32, tag="e4", bufs=2)
                    nc.vector.tensor_scalar(e4[:], mx[:, 0:4], mx[:, 0:1], None,
                                            OP.subtract)
                    nc.scalar.activation(e4[:], e4[:], AF.Exp)
                    s4 = rp.tile([P, 1], f32, tag="s4", bufs=2)
                    nc.vector.reduce_sum(s4[:], e4[:], axis=AX.X)
                    nc.scalar.activation(s4[:], s4[:], AF.Ln)
                    nc.vector.tensor_add(s4[:], s4[:], mx[:, 0:1])
                    nc.vector.tensor_scalar_mul(pk[:, 2 * sch:2 * sch + 1], s4[:], -1.0)
                    mid = rp.tile([P, 1], f32, tag="mid", bufs=2)
                    nc.vector.tensor_add(mid[:], mx[:, 3:4], mx[:, 4:5])
                    nc.vector.tensor_scalar_mul(pk[:, 2 * sch + 1:2 * sch + 2],
                                                mid[:], 0.5)
                tp_ps = ps_sm.tile([16, 512], f32, tag="small", bufs=2)
                nc.tensor.transpose(tp_ps[:16, :P], pk[:], ident[:])
                t16 = rp.tile([16, P], f32, tag="t16sb")
                nc.vector.tensor_copy(t16[:], tp_ps[:16, :P])
                dsc = dpool.tile([16, P], f32, tag="dscr")
                nc.sync.dma_start(dsc[:], t16[:])
                brow = rp.tile([1, S], f32, tag="brow")
                mrow = rp.tile([1, S], f32, tag="mrow")
                for sch in range(SCH):
                    nc.sync.dma_start(brow[:, sch * P:(sch + 1) * P],
                                      dsc[2 * sch:2 * sch + 1, :])
                    nc.sync.dma_start(mrow[:, sch * P:(sch + 1) * P],
                                      dsc[2 * sch + 1:2 * sch + 2, :])
                bias_b = rp.tile([P, S], bf16, tag="biasb")
                mid_b = rp.tile([P, S], bf16, tag="midb")
                for sc in range(SC):
                    sl = slice(sc * 512, (sc + 1) * 512)
                    ps = pmm()
                    nc.tensor.matmul(ps[:], ones_r[:, :P], brow[:, sl],
                                     start=True, stop=True)
                    nc.vector.tensor_copy(bias_b[:, sl], ps[:])
                    ps2 = pmm()
                    nc.tensor.matmul(ps2[:], ones_r[:, :P], mrow[:, sl],
                                     start=True, stop=True)
                    nc.vector.tensor_copy(mid_b[:, sl], ps2[:])
                pt = rp.tile([P, 2, S], bf16, tag="PT")
                for mch in range(2):
                    for sc in range(SC):
                        sl = slice(sc * 512, (sc + 1) * 512)
                        ps = pmm()
                        for k in range(KD):
                            nc.tensor.matmul(ps[:], phiT_b[:, k, mch * P:(mch + 1) * P],
                                             qrT[:, k, sl],
                                             start=(k == 0), stop=(k == KD - 1))
                        t1 = rp.tile([P, 512], f32, tag="ptt1", bufs=2)
                        nc.vector.tensor_add(t1[:], ps[:], bias_b[:, sl])
                        eb = rp.tile([P, 512], bf16, tag="pte", bufs=2)
                        nc.scalar.activation(eb[:], t1[:], AF.Exp)
                        gb = rp.tile([P, 512], bf16, tag="ptg", bufs=2)
                        nc.vector.tensor_tensor(gb[:], ps[:], mid_b[:, sl], op=OP.is_ge)
                        nc.vector.tensor_mul(pt[:, mch, sl], eb[:], gb[:])
                routed = rp.tile([P, KD, S], bf16, tag="routedT")
                for m in range(KD):
                    for sc in range(SC):
                        sl = slice(sc * 512, (sc + 1) * 512)
                        ps = pmm()
                        for k in range(2):
                            nc.tensor.matmul(ps[:], z_sb[:, k, m * P:(m + 1) * P],
                                             pt[:, k, sl],
                                             start=(k == 0), stop=(k == 1))
                        nc.vector.tensor_add(routed[:, m, sl], ps[:], h_sb[:, m, sl])
                out_sb = rp.tile([P, 2, S], f32, tag="outT")
                for m in range(2):
                    for sc in range(SC):
                        sl = slice(sc * 512, (sc + 1) * 512)
                        ps = pmm()
                        for k in range(KD):
                            nc.tensor.matmul(ps[:], wout_w[:, k, m * P:(m + 1) * P],
                                             routed[:, k, sl],
                                             start=(k == 0), stop=(k == KD - 1))
                        if flags["bout"]:
                            nc.vector.tensor_scalar_add(out_sb[:, m, sl], ps[:],
                                                        bout_c[:, m:m + 1])
                        else:
                            nc.vector.tensor_copy(out_sb[:, m, sl], ps[:])
                if reps == 1:
                    nc.sync.dma_start(out_d.rearrange("(o p) s -> p o s", p=P),
                                      out_sb[:])
                else:
                    # timing builds: accumulate so repeated bodies stay live
                    nc.gpsimd.dma_start(out_d.rearrange("(o p) s -> p o s", p=P),
                                        out_sb[:], accum_op=OP.add)

            def body():
                with tc.tile_pool(name="bank", bufs=1) as bp:
                    bank_phase(bp)
                with tc.tile_pool(name="inproj", bufs=1) as ip:
                    h_sb = inproj_phase(ip)
                with tc.tile_pool(name="enc", bufs=1) as ep:
                    for l in range(L):
                        h_sb = encoder_layer(ep, l, h_sb)
                with tc.tile_pool(name="router", bufs=1) as rp:
                    router_phase(rp, h_sb)

            for _ in range(reps):
                body()

    # Pin every activation to the one table set containing Exp+Ln+Square so
    # the table-load pass emits a single load instead of thrashing between
    # exp_and_others and natural_log (~2.7us per switch).
    import concourse.bacc as bacc_mod
    import concourse.hw_specs as hw_specs_mod
    orig = bacc_mod.get_activation_tables
    keepset = "natural_log_exp_and_others"

    def pinned(arch):
        return {k: (v if k == keepset else set())
                for k, v in hw_specs_mod.get_activation_tables(arch).items()}

    bacc_mod.get_activation_tables = pinned
    try:
        nc.compile()
    finally:
        bacc_mod.get_activation_tables = orig
    return nc


def _flags_from(inputs):
    nz = lambda a: bool(np.any(np.asarray(a)))
    return {
        "bqkv": nz(inputs["enc_bqkv"]),
        "bo": nz(inputs["enc_bo"]),
        "b1": nz(inputs["ff_b1"]),
        "b2": nz(inputs["ff_b2"]),
        "ln": (nz(inputs["ln1_b"]) or nz(inputs["ln2_b"])
               or nz(np.asarray(inputs["ln1_g"]) - 1.0)
               or nz(np.asarray(inputs["ln2_g"]) - 1.0)),
        "bout": nz(inputs["b_out"]),
        "mask": not bool(np.all(np.asarray(inputs["mask"]))),
    }


def _pe_table():
    pos = np.arange(S, dtype=np.float32)[:, None]
    div = np.exp(np.arange(0, D, 2, dtype=np.float32) * (-np.log(10000.0) / D))
    pe = np.zeros((S, D), np.float32)
    pe[:, 0::2] = np.sin(pos * div)
    pe[:, 1::2] = np.cos(pos * div)
    return pe


def make_in_maps(inputs):
    f = np.float32
    a = {k: np.asarray(v) for k, v in inputs.items()}
    peT = np.ascontiguousarray(_pe_table().T).astype(ml_dtypes.bfloat16)
    flags = _flags_from(a)
    shared = {
        "peT": peT,
        "win": a["Win"].astype(f), "wout": a["Wout"].astype(f),
        "wqkv": a["enc_Wqkv"].astype(f), "wo4": a["enc_Wo"].astype(f),
        "w1": a["ff_W1"].astype(f), "w2": a["ff_W2"].astype(f),
        "saq": a["sa_Wq"].astype(f), "sak": a["sa_Wk"].astype(f),
        "sav": a["sa_Wv"].astype(f), "sao": a["sa_Wo"].astype(f),
        "rtq": a["rt_Wq"].astype(f),
    }
    if flags["bo"]:
        shared["bo4"] = a["enc_bo"].astype(f)
    if flags["b1"]:
        shared["b14"] = a["ff_b1"].astype(f)
    if flags["b2"]:
        shared["b24"] = a["ff_b2"].astype(f)
    if flags["ln"]:
        shared["lnp"] = np.stack(
            [a["ln1_g"], a["ln1_b"], a["ln2_g"], a["ln2_b"]], axis=1).astype(f)
    if flags["bout"]:
        shared["bout"] = a["b_out"].astype(f)
    maps = []
    for b in range(B):
        m = dict(shared)
        m["xT"] = np.ascontiguousarray(a["x_t"][b].T.astype(f))
        m["temb"] = (a["t_embed"][b] + a["b_in"]).astype(f)
        m["phiT"] = np.ascontiguousarray(a["Phi"][b].T.astype(f))
        m["phi"] = np.ascontiguousarray(a["Phi"][b].astype(f))
        m["sig"] = np.ascontiguousarray(a["Sig"][b].astype(f))
        m["size2"] = np.ascontiguousarray(
            a["Size"][b].astype(f).reshape(M // P, P).T)
        maps.append(m)
    return maps, flags


def get_nc(flags, reps=1):
    key = (tuple(sorted(flags.items())), reps)
    if key not in _CACHE:
        _CACHE[key] = _build(flags, reps)
    return _CACHE[key]


def kernel(**inputs):
    from concourse.bass_utils import run_bass_kernel_spmd
    maps, flags = make_in_maps(inputs)
    nc = get_nc(flags, reps=1)
    res = run_bass_kernel_spmd(nc, maps, list(range(B)))
    out = np.stack([np.ascontiguousarray(res.results[b]["outT"].T)
                    for b in range(B)])
    return out.astype(np.float32)


# revision 17
# speedup vs baseline: 1.0142x; 1.0142x over previous
"""Trainium2 Bass kernel for nn_BankedDenoiser.

Data-parallel over batch: 8 batch elements -> 8 NeuronCores, one element per
core, no collectives. Activations are kept feature-major (hT [D, S]) in SBUF so
every y = x @ W becomes matmul(lhsT=W_chunk, rhs=hT) with weights in their
natural DRAM layout. Matmuls run in bf16 with fp32 PSUM accumulation.

Attention: scores are produced transposed (scoresT [key, query]) so the
exponentiated weights can be used directly as the A@V matmul rhs; the softmax
denominator comes from augmenting token-major V with a ones column (PSUM row
64), and the per-query normalization is applied via a rank-1 ones-broadcast
matmul of the reciprocal row. exp() runs without max-subtraction (scores are
bounded for this module's weight scale) straight out of PSUM on ScalarE.

LayerNorm (feature-major): sum / sum-of-squares via ones-column matmuls on the
PE, rstd = exp(-0.5 * ln(var/D + eps)) so the whole kernel only needs the
natural_log_exp activation table set; mean/rstd rows broadcast with rank-1
matmuls.

Router top-4: Max8 gives each token's top-8 logits sorted; the dense dispatch
matrix P^T[m, s] = exp(logit - m1 - ln(sum_top4_exp)) * (logit >= (m4+m5)/2)
is built in transposed layout and applied as a matmul against token-major Z.
"""

import numpy as np
import ml_dtypes

B, S, IN_DIM, D, H, L, M, TOPK = 8, 1024, 256, 512, 8, 4, 256, 4
DFF = 2048
DH = D // H
TAU, GAMMA, BETA, ETA = 1.0, 0.3, 1.0, 1.0
P = 128
KD = D // P          # 4 feature chunks of 128
SC = S // 512        # 2 column chunks of 512
SCH = S // P         # 8 token chunks of 128
NF = DFF // P        # 16 dff chunks
C2 = BETA / (TAU * D)

_CACHE = {}


def _build(flags, reps=1):
    import concourse.tile as tile
    from concourse import bacc, mybir
    from concourse.masks import make_identity

    f32 = mybir.dt.float32
    bf16 = mybir.dt.bfloat16
    AF = mybir.ActivationFunctionType
    OP = mybir.AluOpType
    AX = mybir.AxisListType

    assert not flags["mask"], "non-trivial mask not supported"
    assert not flags["bqkv"], "nonzero enc qkv bias not supported"

    nc = bacc.Bacc("TRN2", target_bir_lowering=False, debug=False, num_devices=8)

    def din(name, shape, dt=f32):
        return nc.dram_tensor(name, shape, dt, kind="ExternalInput").ap()

    xT_d = din("xT", [IN_DIM, S])
    temb_d = din("temb", [D])           # t_embed[b] + b_in (host-folded)
    peT_d = din("peT", [D, S], bf16)
    phiT_d = din("phiT", [D, M])
    phi_d = din("phi", [M, D])
    sig_d = din("sig", [M, D])
    size_d = din("size2", [P, M // P])
    win_d = din("win", [IN_DIM, D])
    wqkv_d = din("wqkv", [L, D, 3 * D])
    wo_d = din("wo4", [L, D, D])
    w1_d = din("w1", [L, D, DFF])
    w2_d = din("w2", [L, DFF, D])
    saq_d = din("saq", [D, D])
    sak_d = din("sak", [D, D])
    sav_d = din("sav", [D, D])
    sao_d = din("sao", [D, D])
    rtq_d = din("rtq", [D, D])
    wout_d = din("wout", [D, IN_DIM])
    bo_d = din("bo4", [L, D]) if flags["bo"] else None
    b1_d = din("b14", [L, DFF]) if flags["b1"] else None
    b2_d = din("b24", [L, D]) if flags["b2"] else None
    ln_d = din("lnp", [L, 4, D]) if flags["ln"] else None
    bout_d = din("bout", [IN_DIM]) if flags["bout"] else None
    out_d = nc.dram_tensor("outT", [IN_DIM, S], f32, kind="ExternalOutput").ap()

    with tile.TileContext(nc) as tc:
        with (tc.tile_pool(name="const", bufs=1) as cpool,
              tc.tile_pool(name="stage", bufs=2) as stage,
              tc.tile_pool(name="keep", bufs=1) as keep,
              tc.tile_pool(name="psmm", bufs=1, space="PSUM") as ps_mm,
              tc.tile_pool(name="pswide", bufs=2, space="PSUM") as ps_wide,
              tc.tile_pool(name="psav", bufs=2, space="PSUM") as ps_av,
              tc.tile_pool(name="pssm", bufs=1, space="PSUM") as ps_sm,
              tc.tile_pool(name="dram", bufs=2, space="DRAM") as dpool):

            ident = cpool.tile([P, P], f32)
            make_identity(nc, ident[:])
            ones_r = cpool.tile([1, D], f32)
            nc.vector.memset(ones_r[:], 1.0)
            ones_cf = cpool.tile([P, 1], f32)
            nc.vector.memset(ones_cf[:], 1.0)
            ones_cb = cpool.tile([P, 1], bf16)
            nc.vector.memset(ones_cb[:], 1.0)
            eps6_c = cpool.tile([P, 1], f32)
            nc.vector.memset(eps6_c[:], 1e-6)
            eps5_r = cpool.tile([1, 1], f32)
            nc.vector.memset(eps5_r[:], 1e-5)

            _pmm_n = [0]

            def pmm():
                _pmm_n[0] += 1
                return ps_mm.tile([P, 512], f32, tag="mm", bufs=1,
                                  name=f"mmps{_pmm_n[0]}")

            def pwide():
                _pmm_n[0] += 1
                return ps_wide.tile([P, 1024], f32, tag="wide", bufs=2,
                                    name=f"wps{_pmm_n[0]}")

            # DRAM fp32 [K, N] -> SBUF bf16 [P, K//P, N] (via fp32 staging)
            def load_w(pool, dram2d, K, N, tag, scale=None):
                ko_n = K // P
                chunk = max(1, min(ko_n, 2048 // N))
                w = pool.tile([P, ko_n, N], bf16, tag=tag)
                src = dram2d.rearrange("(ko p) m -> p ko m", p=P)
                for c0 in range(0, ko_n, chunk):
                    cn = min(chunk, ko_n - c0)
                    st = stage.tile([P, 2048], f32, tag="stage", bufs=1)
                    stv = st[:, :cn * N].rearrange("p (c m) -> p c m", c=cn)
                    nc.sync.dma_start(stv, src[:, c0:c0 + cn, :])
                    if scale is None:
                        nc.gpsimd.tensor_copy(w[:, c0:c0 + cn, :], stv)
                    else:
                        nc.gpsimd.tensor_scalar_mul(w[:, c0:c0 + cn, :], stv, scale)
                return w

            def col_from(dram1d, n, tag):
                t = cpool.tile([P, n // P], f32, tag=tag)
                nc.sync.dma_start(t[:], dram1d.rearrange("(o p) -> p o", p=P))
                return t

            if flags["bo"]:
                bo_c = [col_from(bo_d[l], D, f"bo{l}") for l in range(L)]
            if flags["b1"]:
                b1_c = [col_from(b1_d[l], DFF, f"b1{l}") for l in range(L)]
            if flags["b2"]:
                b2_c = [col_from(b2_d[l], D, f"b2{l}") for l in range(L)]
            if flags["ln"]:
                ln_c = [[col_from(ln_d[l, j], D, f"ln{l}_{j}") for j in range(4)]
                        for l in range(L)]
            if flags["bout"]:
                bout_c = col_from(bout_d, IN_DIM, "boutc")
            temb_c = col_from(temb_d, D, "tembc")

            # persistent across phases
            phiT_b = keep.tile([P, KD, M], bf16, tag="phiTb")
            z_sb = keep.tile([P, 2, D], bf16, tag="ztok")

            def bank_phase(bp):
                saq_w = load_w(bp, saq_d, D, D, "saq")
                sak_w = load_w(bp, sak_d, D, D, "sak",
                               scale=BETA * ETA / np.sqrt(DH))
                sav_w = load_w(bp, sav_d, D, D, "sav")
                sao_w = load_w(bp, sao_d, D, D, "sao")

                st = stage.tile([P, 2048], f32, tag="stage", bufs=1)
                phiT_f = st[:, :KD * M].rearrange("p (ko m) -> p ko m", ko=KD)
                nc.sync.dma_start(phiT_f, phiT_d.rearrange("(ko p) m -> p ko m", p=P))
                nc.gpsimd.tensor_copy(phiT_b[:], phiT_f)
                phiT_2c = bp.tile([P, KD, M], bf16, tag="phiT2c")
                nc.gpsimd.tensor_scalar_mul(phiT_2c[:], phiT_f, 2.0 * C2)

                phi_sb = bp.tile([P, 2, D], f32, tag="phitok")
                nc.sync.dma_start(phi_sb[:], phi_d.rearrange("(c p) d -> p c d", p=P))
                sig_sb = bp.tile([P, 2, D], f32, tag="sigtok")
                nc.sync.dma_start(sig_sb[:], sig_d.rearrange("(c p) d -> p c d", p=P))
                size_sb = bp.tile([P, 2], f32, tag="sizec")
                nc.sync.dma_start(size_sb[:], size_d[:])

                p2_c = bp.tile([P, 2], f32, tag="p2c")
                sig_c = bp.tile([P, 2], f32, tag="sigc")
                for c in range(2):
                    sq = bp.tile([P, D], bf16, tag="banksq", bufs=2)
                    nc.scalar.activation(sq[:], phi_sb[:, c, :], AF.Square)
                    nc.vector.reduce_sum(p2_c[:, c:c + 1], sq[:], axis=AX.X)
                    sq2 = bp.tile([P, D], bf16, tag="banksq", bufs=2)
                    nc.scalar.activation(sq2[:], sig_sb[:, c, :], AF.Square)
                    nc.vector.reduce_sum(sig_c[:, c:c + 1], sq2[:], axis=AX.X)
                lnsz = bp.tile([P, 2], f32, tag="lnsz")
                nc.scalar.activation(lnsz[:], size_sb[:], AF.Ln, bias=eps6_c[:])
                d_col = bp.tile([P, 2], f32, tag="dcol")
                t_col = bp.tile([P, 2], f32, tag="tcol")
                nc.vector.tensor_scalar_mul(d_col[:], lnsz[:], GAMMA)
                nc.vector.tensor_scalar_mul(t_col[:], sig_c[:], 0.5 / D)
                nc.vector.tensor_sub(d_col[:], d_col[:], t_col[:])
                nc.vector.tensor_scalar_mul(t_col[:], p2_c[:], C2)
                nc.vector.tensor_sub(d_col[:], d_col[:], t_col[:])
                p2m_c = bp.tile([P, 2], f32, tag="p2mc")
                nc.vector.tensor_scalar_mul(p2m_c[:], p2_c[:], -C2)
                pack = bp.tile([P, 4], f32, tag="pack4")
                nc.vector.tensor_copy(pack[:, 0:2], d_col[:])
                nc.vector.tensor_copy(pack[:, 2:4], p2m_c[:])
                tp_ps = ps_sm.tile([16, 512], f32, tag="small", bufs=1)
                nc.tensor.transpose(tp_ps[:4, :P], pack[:], ident[:])
                t4 = bp.tile([4, P], f32, tag="t4sb")
                nc.vector.tensor_copy(t4[:], tp_ps[:4, :P])
                dsc = dpool.tile([16, P], f32, tag="dscr")
                nc.sync.dma_start(dsc[0:4, :], t4[:])
                d_row = bp.tile([1, M], f32, tag="drow")
                p2m_row = bp.tile([1, M], f32, tag="p2mrow")
                for c in range(2):
                    nc.sync.dma_start(d_row[:, c * P:(c + 1) * P], dsc[c:c + 1, :])
                    nc.sync.dma_start(p2m_row[:, c * P:(c + 1) * P],
                                      dsc[2 + c:3 + c, :])

                qTb = bp.tile([P, KD, M], bf16, tag="qTb")
                kTb = bp.tile([P, KD, M], bf16, tag="kTb")
                for dst, wmat in ((qTb, saq_w), (kTb, sak_w)):
                    for m in range(KD):
                        ps = pmm()
                        for k in range(KD):
                            nc.tensor.matmul(ps[:, :M],
                                             wmat[:, k, m * P:(m + 1) * P],
                                             phiT_b[:, k, :],
                                             start=(k == 0), stop=(k == KD - 1))
                        nc.vector.tensor_copy(dst[:, m, :], ps[:, :M])
                vb_aug = bp.tile([P, 2, H, DH + 1], bf16, tag="vbaug")
                nc.vector.memset(vb_aug[:], 1.0)
                for nch in range(2):
                    ps = pmm()
                    for k in range(KD):
                        nc.tensor.matmul(ps[:], phiT_b[:, k, nch * P:(nch + 1) * P],
                                         sav_w[:, k, :],
                                         start=(k == 0), stop=(k == KD - 1))
                    nc.vector.tensor_copy(
                        vb_aug[:, nch, :, 0:DH],
                        ps[:].rearrange("p (h c) -> p h c", c=DH))

                oTb = bp.tile([P, KD, M], bf16, tag="oTb")
                for h in range(H):
                    p0, ko = DH * (h % 2), h // 2
                    eb = bp.tile([P, 2, M], bf16, tag="expb", bufs=2)
                    for nch in range(2):
                        ps = pmm()
                        for k in range(KD):
                            nc.tensor.matmul(ps[:, :M],
                                             phiT_b[:, k, nch * P:(nch + 1) * P],
                                             phiT_2c[:, k, :],
                                             start=(k == 0), stop=False)
                        nc.tensor.matmul(ps[:, :M], d_row[:, nch * P:(nch + 1) * P],
                                         ones_r[:, :M], start=False, stop=False)
                        nc.tensor.matmul(ps[:, :M], ones_r[:, :P], p2m_row[:],
                                         start=False, stop=False)
                        nc.tensor.matmul(ps[:, :M],
                                         kTb[p0:p0 + DH, ko, nch * P:(nch + 1) * P],
                                         qTb[p0:p0 + DH, ko, :],
                                         start=False, stop=True)
                        nc.scalar.activation(eb[:, nch, :], ps[:, :M], AF.Exp)
                    zb = ps_av.tile([DH + 1, 512], f32, tag="av", bufs=2)
                    for nch in range(2):
                        nc.tensor.matmul(zb[:, :M], vb_aug[:, nch, h, :],
                                         eb[:, nch, :],
                                         start=(nch == 0), stop=(nch == 1))
                    den = bp.tile([1, M], f32, tag="denb", bufs=2)
                    nc.vector.tensor_copy(den[:], zb[DH:DH + 1, :M])
                    rb = bp.tile([1, M], f32, tag="recb", bufs=2)
                    nc.vector.reciprocal(rb[:], den[:])
                    bc = ps_sm.tile([DH, 512], f32, tag="small", bufs=1)
                    nc.tensor.matmul(bc[:, :M], ones_r[:, :DH], rb[:],
                                     start=True, stop=True)
                    bcs = bp.tile([DH, M], bf16, tag="bcsb", bufs=2)
                    nc.vector.tensor_copy(bcs[:], bc[:, :M])
                    nc.vector.tensor_mul(oTb[p0:p0 + DH, ko, :], zb[0:DH, :M], bcs[:])
                for mch in range(2):
                    ps = pmm()
                    for k in range(KD):
                        nc.tensor.matmul(ps[:], oTb[:, k, mch * P:(mch + 1) * P],
                                         sao_w[:, k, :],
                                         start=(k == 0), stop=(k == KD - 1))
                    nc.vector.tensor_copy(z_sb[:, mch, :], ps[:])

            def inproj_phase(ip):
                win_w = load_w(ip, win_d, IN_DIM, D, "win")
                peT_sb = ip.tile([P, KD, S], bf16, tag="peT")
                nc.sync.dma_start(peT_sb[:],
                                  peT_d.rearrange("(ko p) s -> p ko s", p=P))
                st = stage.tile([P, 2048], f32, tag="stage", bufs=1)
                xT_f = st[:, :2 * 1024].rearrange("p (ko s) -> p ko s", ko=2)
                nc.sync.dma_start(xT_f, xT_d.rearrange("(ko p) s -> p ko s", p=P))
                xT_b = ip.tile([P, 2, S], bf16, tag="xTb")
                nc.gpsimd.tensor_copy(xT_b[:], xT_f)
                h_sb = keep.tile([P, KD, S], bf16, tag="hT", bufs=2)
                for m in range(KD):
                    for sc in range(SC):
                        sl = slice(sc * 512, (sc + 1) * 512)
                        ps = pmm()
                        for k in range(2):
                            nc.tensor.matmul(ps[:], win_w[:, k, m * P:(m + 1) * P],
                                             xT_b[:, k, sl],
                                             start=(k == 0), stop=(k == 1))
                        tmp = ip.tile([P, 512], bf16, tag="iptmp", bufs=2)
                        nc.vector.tensor_add(tmp[:], ps[:], peT_sb[:, m, sl])
                        nc.vector.tensor_scalar_add(h_sb[:, m, sl], tmp[:],
                                                    temb_c[:, m:m + 1])
                return h_sb

            def emit_ln(ep, r_t, rsq_t, lidx, lnoff):
                # r_t: bf16 [P, KD, S]; rsq_t: bf16 [P, KD, S] (squares)
                out = keep.tile([P, KD, S], bf16, tag="hT", bufs=2)
                for sc in range(SC):
                    sl = slice(sc * 512, (sc + 1) * 512)
                    ps1 = ps_sm.tile([16, 512], f32, tag="small", bufs=1)
                    for k in range(KD):
                        nc.tensor.matmul(ps1[:1, :], ones_cb[:], r_t[:, k, sl],
                                         start=(k == 0), stop=(k == KD - 1))
                    mu_row = ep.tile([1, 512], f32, tag="murow", bufs=2)
                    nc.vector.tensor_scalar_mul(mu_row[:], ps1[:1, :], 1.0 / D)
                    tr = ep.tile([1, 512], f32, tag="tmprow", bufs=2)
                    nc.vector.tensor_mul(tr[:], ps1[:1, :], mu_row[:])
                    ps2 = ps_sm.tile([16, 512], f32, tag="small", bufs=1)
                    for k in range(KD):
                        nc.tensor.matmul(ps2[:1, :], ones_cb[:], rsq_t[:, k, sl],
                                         start=(k == 0), stop=(k == KD - 1))
                    var_row = ep.tile([1, 512], f32, tag="varrow", bufs=2)
                    nc.vector.tensor_sub(var_row[:], ps2[:1, :], tr[:])
                    nc.scalar.activation(var_row[:], var_row[:], AF.Ln,
                                         bias=eps5_r[:], scale=1.0 / D)
                    rstd_row = ep.tile([1, 512], f32, tag="rstdrow", bufs=2)
                    nc.scalar.activation(rstd_row[:], var_row[:], AF.Exp, scale=-0.5)
                    mb_ps = pmm()
                    nc.tensor.matmul(mb_ps[:], ones_r[:, :P], mu_row[:],
                                     start=True, stop=True)
                    mb = ep.tile([P, 512], bf16, tag="mubc", bufs=2)
                    nc.vector.tensor_copy(mb[:], mb_ps[:])
                    rb_ps = pmm()
                    nc.tensor.matmul(rb_ps[:], ones_r[:, :P], rstd_row[:],
                                     start=True, stop=True)
                    rbt = ep.tile([P, 512], bf16, tag="rstdbc", bufs=2)
                    nc.vector.tensor_copy(rbt[:], rb_ps[:])
                    for k in range(KD):
                        t1 = ep.tile([P, 512], bf16, tag="lnt1", bufs=2)
                        nc.vector.tensor_sub(t1[:], r_t[:, k, sl], mb[:])
                        if flags["ln"]:
                            t2 = ep.tile([P, 512], bf16, tag="lnt2", bufs=2)
                            nc.vector.tensor_mul(t2[:], t1[:], rbt[:])
                            nc.vector.tensor_scalar(
                                out[:, k, sl], t2[:],
                                ln_c[lidx][lnoff][:, k:k + 1],
                                ln_c[lidx][lnoff + 1][:, k:k + 1], OP.mult, OP.add)
                        else:
                            nc.vector.tensor_mul(out[:, k, sl], t1[:], rbt[:])
                return out

            def encoder_layer(ep, l, h_sb):
                wqkv_w = load_w(ep, wqkv_d[l], D, 3 * D, "wqkv")
                wo_w = load_w(ep, wo_d[l], D, D, "wo")
                w1_w = load_w(ep, w1_d[l], D, DFF, "w1")
                w2_w = load_w(ep, w2_d[l], DFF, D, "w2")
                qT = ep.tile([P, KD, S], bf16, tag="qT")
                kT = ep.tile([P, KD, S], bf16, tag="kT")
                for which, dst in ((0, qT), (1, kT)):
                    off = which * D
                    for m in range(KD):
                        ps = pwide()
                        for sc in range(SC):
                            sl = slice(sc * 512, (sc + 1) * 512)
                            for k in range(KD):
                                nc.tensor.matmul(
                                    ps[:, sl], wqkv_w[:, k, off + m * P:off + (m + 1) * P],
                                    h_sb[:, k, sl],
                                    start=(k == 0), stop=(k == KD - 1))
                        if which == 0:
                            nc.vector.tensor_scalar_mul(dst[:, m, :], ps[:],
                                                        1.0 / np.sqrt(DH))
                        else:
                            nc.scalar.copy(dst[:, m, :], ps[:])
                v_aug = ep.tile([P, SCH, H, DH + 1], bf16, tag="vaug")
                nc.vector.memset(v_aug[:], 1.0)
                for tch in range(SCH):
                    ps = pmm()
                    for k in range(KD):
                        nc.tensor.matmul(ps[:], h_sb[:, k, tch * P:(tch + 1) * P],
                                         wqkv_w[:, k, 2 * D:3 * D],
                                         start=(k == 0), stop=(k == KD - 1))
                    nc.scalar.copy(
                        v_aug[:, tch, :, 0:DH],
                        ps[:].rearrange("p (h c) -> p h c", c=DH))
                oT = ep.tile([P, KD, S], bf16, tag="oT")
                # heads 2*ko (partitions 0:64) and 2*ko+1 (64:128) interleave so
                # their K=64 score matmuls pack into disjoint PE row groups
                for ko in range(KD):
                    ets = [[], []]
                    for tch in range(SCH):
                        for hp in range(2):
                            p0 = DH * hp
                            ps = pwide()
                            for sc in range(SC):
                                sl = slice(sc * 512, (sc + 1) * 512)
                                nc.tensor.matmul(
                                    ps[:, sl], kT[p0:p0 + DH, ko, tch * P:(tch + 1) * P],
                                    qT[p0:p0 + DH, ko, sl], start=True, stop=True)
                            et = ep.tile([P, 1024], bf16, tag="expT", bufs=16)
                            nc.scalar.activation(et[:], ps[:], AF.Exp)
                            ets[hp].append(et)
                    for hp in range(2):
                        h = 2 * ko + hp
                        p0 = DH * hp
                        for sc in range(SC):
                            sl = slice(sc * 512, (sc + 1) * 512)
                            zo = ps_av.tile([DH + 1, 512], f32, tag="av", bufs=2)
                            for tch in range(SCH):
                                nc.tensor.matmul(zo[:], v_aug[:, tch, h, :],
                                                 ets[hp][tch][:, sl],
                                                 start=(tch == 0), stop=(tch == SCH - 1))
                            den = ep.tile([1, 512], f32, tag="den", bufs=2)
                            nc.vector.tensor_copy(den[:], zo[DH:DH + 1, :])
                            rcp = ep.tile([1, 512], f32, tag="rcp", bufs=2)
                            nc.vector.reciprocal(rcp[:], den[:])
                            bc = ps_sm.tile([DH, 512], f32, tag="small", bufs=1)
                            nc.tensor.matmul(bc[:], ones_r[:, :DH], rcp[:],
                                             start=True, stop=True)
                            bcs = ep.tile([DH, 512], bf16, tag="bcs", bufs=2)
                            nc.vector.tensor_copy(bcs[:], bc[:])
                            nc.vector.tensor_mul(oT[p0:p0 + DH, ko, sl],
                                                 zo[0:DH, :], bcs[:])
                r_t = ep.tile([P, KD, S], bf16, tag="resid")
                rsq_t = ep.tile([P, KD, S], bf16, tag="rsq")
                for m in range(KD):
                    ps = pwide()
                    for sc in range(SC):
                        sl = slice(sc * 512, (sc + 1) * 512)
                        for k in range(KD):
                            nc.tensor.matmul(ps[:, sl], wo_w[:, k, m * P:(m + 1) * P],
                                             oT[:, k, sl],
                                             start=(k == 0), stop=(k == KD - 1))
                    if flags["bo"]:
                        nc.vector.tensor_scalar_add(ps[:], ps[:], bo_c[l][:, m:m + 1])
                    nc.vector.tensor_add(r_t[:, m, :], ps[:], h_sb[:, m, :])
                    nc.scalar.activation(rsq_t[:, m, :], r_t[:, m, :], AF.Square)
                h_sb = emit_ln(ep, r_t, rsq_t, l, 0)
                r_t = ep.tile([P, KD, S], bf16, tag="resid")
                rsq_t = ep.tile([P, KD, S], bf16, tag="rsq")
                for sc in range(SC):
                    sl = slice(sc * 512, (sc + 1) * 512)
                    ff = ep.tile([P, NF, 512], bf16, tag="ffT")
                    for m in range(0, NF, 2):
                        ps = pwide()
                        for j in range(2):
                            half = slice(j * 512, (j + 1) * 512)
                            for k in range(KD):
                                nc.tensor.matmul(
                                    ps[:, half], w1_w[:, k, (m + j) * P:(m + j + 1) * P],
                                    h_sb[:, k, sl],
                                    start=(k == 0), stop=(k == KD - 1))
                        psv = ps[:].rearrange("p (c s) -> p c s", c=2)
                        if flags["b1"]:
                            nc.vector.tensor_scalar(ff[:, m:m + 2, :], psv,
                                                    b1_c[l][:, m:m + 1], 0.0,
                                                    OP.add, OP.max)
                        else:
                            nc.vector.tensor_scalar_max(ff[:, m:m + 2, :], psv, 0.0)
                    for m in range(0, KD, 2):
                        ps = pwide()
                        for j in range(2):
                            half = slice(j * 512, (j + 1) * 512)
                            for k in range(NF):
                                nc.tensor.matmul(
                                    ps[:, half], w2_w[:, k, (m + j) * P:(m + j + 1) * P],
                                    ff[:, k, :],
                                    start=(k == 0), stop=(k == NF - 1))
                        psv = ps[:].rearrange("p (c s) -> p c s", c=2)
                        if flags["b2"]:
                            nc.vector.tensor_scalar_add(psv, psv, b2_c[l][:, m:m + 1])
                        nc.vector.tensor_add(r_t[:, m:m + 2, sl], psv,
                                             h_sb[:, m:m + 2, sl])
                        nc.scalar.activation(rsq_t[:, m:m + 2, sl],
                                             r_t[:, m:m + 2, sl], AF.Square)
                return emit_ln(ep, r_t, rsq_t, l, 2)

            def router_phase(rp, h_sb):
                rtq_w = load_w(rp, rtq_d, D, D, "rtq", scale=1.0 / np.sqrt(D))
                wout_w = load_w(rp, wout_d, D, IN_DIM, "wout")
                qrT = rp.tile([P, KD, S], bf16, tag="qrT")
                for m in range(KD):
                    for sc in range(SC):
                        sl = slice(sc * 512, (sc + 1) * 512)
                        ps = pmm()
                        for k in range(KD):
                            nc.tensor.matmul(ps[:], rtq_w[:, k, m * P:(m + 1) * P],
                                             h_sb[:, k, sl],
                                             start=(k == 0), stop=(k == KD - 1))
                        nc.vector.tensor_copy(qrT[:, m, sl], ps[:])
                pk = rp.tile([P, 16], f32, tag="pk")
                for sch in range(SCH):
                    ps = pmm()
                    for k in range(KD):
                        nc.tensor.matmul(ps[:, :M], qrT[:, k, sch * P:(sch + 1) * P],
                                         phiT_b[:, k, :],
                                         start=(k == 0), stop=(k == KD - 1))
                    lg = rp.tile([P, M], f32, tag="lgtok", bufs=2)
                    nc.vector.tensor_copy(lg[:], ps[:, :M])
                    mx = rp.tile([P, 8], f32, tag="mx8", bufs=2)
                    nc.vector.max(mx[:], lg[:])
                    e4 = rp.tile([P, 4], f32, tag="e4", bufs=2)
                    nc.vector.tensor_scalar(e4[:], mx[:, 0:4], mx[:, 0:1], None,
                                            OP.subtract)
                    nc.scalar.activation(e4[:], e4[:], AF.Exp)
                    s4 = rp.tile([P, 1], f32, tag="s4", bufs=2)
                    nc.vector.reduce_sum(s4[:], e4[:], axis=AX.X)
                    nc.scalar.activation(s4[:], s4[:], AF.Ln)
                    nc.vector.tensor_add(s4[:], s4[:], mx[:, 0:1])
                    nc.vector.tensor_scalar_mul(pk[:, 2 * sch:2 * sch + 1], s4[:], -1.0)
                    mid = rp.tile([P, 1], f32, tag="mid", bufs=2)
                    nc.vector.tensor_add(mid[:], mx[:, 3:4], mx[:, 4:5])
                    nc.vector.tensor_scalar_mul(pk[:, 2 * sch + 1:2 * sch + 2],
                                                mid[:], 0.5)
                tp_ps = ps_sm.tile([16, 512], f32, tag="small", bufs=1)
                nc.tensor.transpose(tp_ps[:16, :P], pk[:], ident[:])
                t16 = rp.tile([16, P], f32, tag="t16sb")
                nc.vector.tensor_copy(t16[:], tp_ps[:16, :P])
                dsc = dpool.tile([16, P], f32, tag="dscr")
                nc.sync.dma_start(dsc[:], t16[:])
                brow = rp.tile([1, S], f32, tag="brow")
                mrow = rp.tile([1, S], f32, tag="mrow")
                for sch in range(SCH):
                    nc.sync.dma_start(brow[:, sch * P:(sch + 1) * P],
                                      dsc[2 * sch:2 * sch + 1, :])
                    nc.sync.dma_start(mrow[:, sch * P:(sch + 1) * P],
                                      dsc[2 * sch + 1:2 * sch + 2, :])
                bias_b = rp.tile([P, S], bf16, tag="biasb")
                mid_b = rp.tile([P, S], bf16, tag="midb")
                for sc in range(SC):
                    sl = slice(sc * 512, (sc + 1) * 512)
                    ps = pmm()
                    nc.tensor.matmul(ps[:], ones_r[:, :P], brow[:, sl],
                                     start=True, stop=True)
                    nc.vector.tensor_copy(bias_b[:, sl], ps[:])
                    ps2 = pmm()
                    nc.tensor.matmul(ps2[:], ones_r[:, :P], mrow[:, sl],
                                     start=True, stop=True)
                    nc.vector.tensor_copy(mid_b[:, sl], ps2[:])
                pt = rp.tile([P, 2, S], bf16, tag="PT")
                for mch in range(2):
                    for sc in range(SC):
                        sl = slice(sc * 512, (sc + 1) * 512)
                        ps = pmm()
                        for k in range(KD):
                            nc.tensor.matmul(ps[:], phiT_b[:, k, mch * P:(mch + 1) * P],
                                             qrT[:, k, sl],
                                             start=(k == 0), stop=(k == KD - 1))
                        t1 = rp.tile([P, 512], f32, tag="ptt1", bufs=2)
                        nc.vector.tensor_add(t1[:], ps[:], bias_b[:, sl])
                        eb = rp.tile([P, 512], bf16, tag="pte", bufs=2)
                        nc.scalar.activation(eb[:], t1[:], AF.Exp)
                        gb = rp.tile([P, 512], bf16, tag="ptg", bufs=2)
                        nc.vector.tensor_tensor(gb[:], ps[:], mid_b[:, sl], op=OP.is_ge)
                        nc.vector.tensor_mul(pt[:, mch, sl], eb[:], gb[:])
                routed = rp.tile([P, KD, S], bf16, tag="routedT")
                for m in range(KD):
                    for sc in range(SC):
                        sl = slice(sc * 512, (sc + 1) * 512)
                        ps = pmm()
                        for k in range(2):
                            nc.tensor.matmul(ps[:], z_sb[:, k, m * P:(m + 1) * P],
                                             pt[:, k, sl],
                                             start=(k == 0), stop=(k == 1))
                        nc.vector.tensor_add(routed[:, m, sl], ps[:], h_sb[:, m, sl])
                out_sb = rp.tile([P, 2, S], f32, tag="outT")
                for m in range(2):
                    for sc in range(SC):
                        sl = slice(sc * 512, (sc + 1) * 512)
                        ps = pmm()
                        for k in range(KD):
                            nc.tensor.matmul(ps[:], wout_w[:, k, m * P:(m + 1) * P],
                                             routed[:, k, sl],
                                             start=(k == 0), stop=(k == KD - 1))
                        if flags["bout"]:
                            nc.vector.tensor_scalar_add(out_sb[:, m, sl], ps[:],
                                                        bout_c[:, m:m + 1])
                        else:
                            nc.vector.tensor_copy(out_sb[:, m, sl], ps[:])
                if reps == 1:
                    nc.sync.dma_start(out_d.rearrange("(o p) s -> p o s", p=P),
                                      out_sb[:])
                else:
                    # timing builds: accumulate so repeated bodies stay live
                    nc.gpsimd.dma_start(out_d.rearrange("(o p) s -> p o s", p=P),
                                        out_sb[:], accum_op=OP.add)

            def body():
                with tc.tile_pool(name="bank", bufs=1) as bp:
                    bank_phase(bp)
                with tc.tile_pool(name="inproj", bufs=1) as ip:
                    h_sb = inproj_phase(ip)
                with tc.tile_pool(name="enc", bufs=1) as ep:
                    for l in range(L):
                        h_sb = encoder_layer(ep, l, h_sb)
                with tc.tile_pool(name="router", bufs=1) as rp:
                    router_phase(rp, h_sb)

            for _ in range(reps):
                body()

    # Pin every activation to the one table set containing Exp+Ln+Square so
    # the table-load pass emits a single load instead of thrashing between
    # exp_and_others and natural_log (~2.7us per switch).
    import concourse.bacc as bacc_mod
    import concourse.hw_specs as hw_specs_mod
    orig = bacc_mod.get_activation_tables
    keepset = "natural_log_exp_and_others"

    def pinned(arch):
        return {k: (v if k == keepset else set())
                for k, v in hw_specs_mod.get_activation_tables(arch).items()}

    bacc_mod.get_activation_tables = pinned
    try:
        nc.compile()
    finally:
        bacc_mod.get_activation_tables = orig
    return nc


def _flags_from(inputs):
    nz = lambda a: bool(np.any(np.asarray(a)))
    return {
        "bqkv": nz(inputs["enc_bqkv"]),
        "bo": nz(inputs["enc_bo"]),
        "b1": nz(inputs["ff_b1"]),
        "b2": nz(inputs["ff_b2"]),
        "ln": (nz(inputs["ln1_b"]) or nz(inputs["ln2_b"])
               or nz(np.asarray(inputs["ln1_g"]) - 1.0)
               or nz(np.asarray(inputs["ln2_g"]) - 1.0)),
        "bout": nz(inputs["b_out"]),
        "mask": not bool(np.all(np.asarray(inputs["mask"]))),
    }


def _pe_table():
    pos = np.arange(S, dtype=np.float32)[:, None]
    div = np.exp(np.arange(0, D, 2, dtype=np.float32) * (-np.log(10000.0) / D))
    pe = np.zeros((S, D), np.float32)
    pe[:, 0::2] = np.sin(pos * div)
    pe[:, 1::2] = np.cos(pos * div)
    return pe


def make_in_maps(inputs):
    f = np.float32
    a = {k: np.asarray(v) for k, v in inputs.items()}
    peT = np.ascontiguousarray(_pe_table().T).astype(ml_dtypes.bfloat16)
    flags = _flags_from(a)
    shared = {
        "peT": peT,
        "win": a["Win"].astype(f), "wout": a["Wout"].astype(f),
        "wqkv": a["enc_Wqkv"].astype(f), "wo4": a["enc_Wo"].astype(f),
        "w1": a["ff_W1"].astype(f), "w2": a["ff_W2"].astype(f),
        "saq": a["sa_Wq"].astype(f), "sak": a["sa_Wk"].astype(f),
        "sav": a["sa_Wv"].astype(f), "sao": a["sa_Wo"].astype(f),
        "rtq": a["rt_Wq"].astype(f),
    }
    if flags["bo"]:
        shared["bo4"] = a["enc_bo"].astype(f)
    if flags["b1"]:
        shared["b14"] = a["ff_b1"].astype(f)
    if flags["b2"]:
        shared["b24"] = a["ff_b2"].astype(f)
    if flags["ln"]:
        shared["lnp"] = np.stack(
            [a["ln1_g"], a["ln1_b"], a["ln2_g"], a["ln2_b"]], axis=1).astype(f)
    if flags["bout"]:
        shared["bout"] = a["b_out"].astype(f)
    maps = []
    for b in range(B):
        m = dict(shared)
        m["xT"] = np.ascontiguousarray(a["x_t"][b].T.astype(f))
        m["temb"] = (a["t_embed"][b] + a["b_in"]).astype(f)
        m["phiT"] = np.ascontiguousarray(a["Phi"][b].T.astype(f))
        m["phi"] = np.ascontiguousarray(a["Phi"][b].astype(f))
        m["sig"] = np.ascontiguousarray(a["Sig"][b].astype(f))
        m["size2"] = np.ascontiguousarray(
            a["Size"][b].astype(f).reshape(M // P, P).T)
        maps.append(m)
    return maps, flags


def get_nc(flags, reps=1):
    key = (tuple(sorted(flags.items())), reps)
    if key not in _CACHE:
        _CACHE[key] = _build(flags, reps)
    return _CACHE[key]


def kernel(**inputs):
    from concourse.bass_utils import run_bass_kernel_spmd
    maps, flags = make_in_maps(inputs)
    nc = get_nc(flags, reps=1)
    res = run_bass_kernel_spmd(nc, maps, list(range(B)))
    out = np.stack([np.ascontiguousarray(res.results[b]["outT"].T)
                    for b in range(B)])
    return out.astype(np.float32)
